# revision 1
# baseline (speedup 1.0000x reference)
"""Trainium2 Bass kernel for BlittingStrokeModel (AA polyline rasterization).

Reference semantics: for each batch item, rasterize 16 AA line segments
(trajectory knots) onto a zero canvas via a point-to-segment distance field:
    dist = point-to-segment distance
    cov  = clip(line_width + 0.5 - dist, 0, 1)
    out  = max over segments, broadcast to 3 channels.

Device formulation (exact up to the reference's 1e-8/1e-12 epsilons). With
s = 1/sqrt(dd2), dd2 = dx^2+dy^2, dn2 = dd2/2:
    w   = (dx*x + dy*y - c0 - dn2) * s        # scaled, recentred dot product
    E   = relu(|w| - dn2*s)                   # segment-clamp excess / sqrt(dd2)
    Pp  = (dy*x - dx*y + cP) * s              # perpendicular line distance
    dist^2 = Pp^2 + E^2
    M   = min over segments of dist^2
    cov = clip(L + 0.5 - sqrt(M), 0, 1)
Max over segments of cov == cov(min dist) since cov is monotone in dist.

Per (segment, 128-row stripe) the engine split is:
    ACT: At = Abs(x*dxs + cdw)     [plane + abs]
    V/ACT: E = relu(At - dn2s)     [assignment balances engine load]
    V:   M' = min((aP*x+bP)^2 + E^2, M)   [one fused custom DVE op; the
         x plane comes from the DVE Idx generator, so Src1 carries M]
Stripes are emitted round-robin with two min-chains each, giving the Tile
scheduler 8 independent chains so no engine starves at the kernel tail.

Input-specialized program structure: host geometry (fp64, conservative
margins) decides per (core, segment, stripe) whether the segment can
influence the stripe at all (skip otherwise) and whether its endpoint-cap
term can matter there (drop the At/E ops and feed E=0 otherwise).  All 8
cores run one SPMD program whose per-stripe slot counts are the max over
cores; cores with fewer jobs pad with neutral coefficients (d2 = 1e12).
Programs are cached per structure; the custom DVE ops are registered at
runtime so this file is self-contained.

Sharding: data-parallel over batch, one image per NeuronCore (8 cores).
The output does not depend on the image *values*, so images never touch
the device; only tiny per-segment coefficient tables are uploaded.
"""

import numpy as np
from contextlib import ExitStack

B, C, H, W = 8, 3, 512, 512
K = 17
NSEG = K - 1
P = 128
NSTRIPE = H // P  # 4
MARG = 1.0  # conservative skip margin in pixels (fp32 error << 1e-2)

_state = {}


# --------------------------------------------------------------------------
# custom DVE ops
# --------------------------------------------------------------------------

def _register_dve_op(name, spec):
    import concourse.dve_ops as dve_ops
    from concourse.dve_ops import DveOp, OPS, _SUB_OPCODE_FOR_NAME, _CUSTOM_DVE_ROW_BASE
    from concourse.dve_spec import lower, _has_src1
    from concourse.dve_uop import DveOpSpec
    from concourse.dve_table_gen import dve_ver_for

    if name in _SUB_OPCODE_FOR_NAME:
        return next(o for o in OPS if o.name == name)
    row = _CUSTOM_DVE_ROW_BASE + len(OPS)
    assert row < 0x20
    _SUB_OPCODE_FOR_NAME[name] = row
    ver = dve_ver_for("TRN2")
    tmp = DveOpSpec(
        name=name, opcode=row, uops=lower(spec, ver=ver), rd1_en=_has_src1(spec)
    )
    op = DveOp(name, spec, subdim=False, uops_sha={ver: tmp.sha(ver)})
    OPS.append(op)
    dve_ops.CUSTOM_DVE_SPECS[name] = spec
    return op


def _get_dve_ops():
    if "ops" in _state:
        return _state["ops"]
    from concourse.dve_spec import (
        Spec, Src0, Src1, C0, C1, sq, minn, maxx, Idx, Zero, One,
    )

    def _idx(in0):
        return np.arange(in0.shape[-1], dtype=np.float32)[None, :]

    d2min = _register_dve_op(
        "STROKE_D2MIN_ANT",
        Spec(
            body=minn(sq(Idx * C0 + C1) + sq(Src0), Src1),
            reference=lambda in0, in1, s0, s1, imm2: np.minimum(
                (_idx(in0) * s0 + s1) ** 2 + in0.astype(np.float32) ** 2, in1
            ).astype(np.float32),
        ),
    )
    d2first = _register_dve_op(
        "STROKE_D2_ANT",
        Spec(
            body=sq(Idx * C0 + C1) + sq(Src0),
            reference=lambda in0, in1, s0, s1, imm2: (
                (_idx(in0) * s0 + s1) ** 2 + in0.astype(np.float32) ** 2
            ).astype(np.float32),
        ),
    )
    clip = _register_dve_op(
        "STROKE_CLIP_ANT",
        Spec(
            body=minn(maxx(C0 - Src0, Zero), One),
            reference=lambda in0, in1, s0, s1, imm2: np.minimum(
                np.maximum(s0 - in0.astype(np.float32), 0.0), 1.0
            ).astype(np.float32),
        ),
    )
    # line-only variants: x comes from Src0 (= xt tile) instead of the Idx
    # scan, so these lower to a single uOp pass (the Idx ops need two)
    lmin = _register_dve_op(
        "STROKE_LD2MIN_ANT",
        Spec(
            body=minn(sq(Src0 * C0 + C1), Src1),
            reference=lambda in0, in1, s0, s1, imm2: np.minimum(
                (in0.astype(np.float32) * s0 + s1) ** 2, in1
            ).astype(np.float32),
        ),
    )
    lfirst = _register_dve_op(
        "STROKE_LD2_ANT",
        Spec(
            body=sq(Src0 * C0 + C1),
            reference=lambda in0, in1, s0, s1, imm2: (
                (in0.astype(np.float32) * s0 + s1) ** 2
            ).astype(np.float32),
        ),
    )
    _state["ops"] = (d2min, d2first, clip, lmin, lfirst)
    return _state["ops"]


# --------------------------------------------------------------------------
# host geometry: which (segment, stripe) pairs can matter, per core
# --------------------------------------------------------------------------

def _segments(xy):
    """Guarded segment endpoints/deltas (fp64). xy: [K, 2]."""
    p0, p1 = xy[:-1].copy(), xy[1:].copy()
    d = p1 - p0
    degen = (d[:, 0] ** 2 + d[:, 1] ** 2) < 1e-12
    d[degen, 0] = 1e-6
    p1 = p0 + d
    return p0, p1, d


def _seg_rect_dist(p0, p1, ylo, yhi):
    """Distance from segment (p0,p1) to rect [0, W-1] x [ylo, yhi]."""
    def pt_in_rect(p):
        return (0.0 <= p[0] <= W - 1) and (ylo <= p[1] <= yhi)

    if pt_in_rect(p0) or pt_in_rect(p1):
        return 0.0

    def ptseg(p, s0, s1):
        d = s1 - s0
        dd = float(d @ d)
        if dd < 1e-18:
            return float(np.hypot(*(p - s0)))
        t = min(1.0, max(0.0, float((p - s0) @ d) / dd))
        return float(np.hypot(*(p - s0 - t * d)))

    def ccw(A, B, C):
        return (C[1] - A[1]) * (B[0] - A[0]) > (B[1] - A[1]) * (C[0] - A[0])

    def inter(A, B, C, D):
        return ccw(A, C, D) != ccw(B, C, D) and ccw(A, B, C) != ccw(A, B, D)

    corners = [
        np.array([0.0, ylo]), np.array([W - 1.0, ylo]),
        np.array([W - 1.0, yhi]), np.array([0.0, yhi]),
    ]
    best = np.inf
    for i in range(4):
        b0, b1 = corners[i], corners[(i + 1) % 4]
        if inter(p0, p1, b0, b1):
            return 0.0
        best = min(
            best,
            ptseg(p0, b0, b1), ptseg(p1, b0, b1),
            ptseg(b0, p0, p1), ptseg(b1, p0, p1),
        )
    return best


def _plan(trajectories, line_width):
    """Decide kept jobs and cap-need per (core, stripe); build the SPMD
    union structure and per-core slot assignments."""
    thr = float(np.asarray(line_width).item()) + 0.5
    xy = np.asarray(trajectories, dtype=np.float64)[:, :, 1:3]
    nb = xy.shape[0]
    R = thr + MARG
    FAR = 1500.0

    # jobs[b][T] = list of (seg, needs_cap) — cap-needing first
    jobs = [[[] for _ in range(NSTRIPE)] for _ in range(nb)]
    for b in range(nb):
        p0a, p1a, da = _segments(xy[b])
        for T in range(NSTRIPE):
            ylo, yhi = T * P + 0.0, T * P + P - 1.0
            full, line = [], []
            for s in range(NSEG):
                p0, p1, d = p0a[s], p1a[s], da[s]
                if _seg_rect_dist(p0, p1, ylo, yhi) > R:
                    continue
                dirv = d / max(float(np.hypot(*d)), 1e-9)
                cap = (
                    _seg_rect_dist(p0, p0 - dirv * FAR, ylo, yhi) <= R
                    or _seg_rect_dist(p1, p1 + dirv * FAR, ylo, yhi) <= R
                )
                (full if cap else line).append((s, cap))
            jobs[b][T] = line + full  # line-only jobs first (no At/E dep)

    # Decouple stripes from images: bin-pack all (image, stripe) pairs
    # across the cores (LPT) so per-core load equalizes — the stripe
    # identity lives entirely in host coefficients + output addressing.
    pairs = sorted(
        (
            (len(jobs[b][T]), sum(1 for _, cp in jobs[b][T] if cp), b, T)
            for b in range(nb)
            for T in range(NSTRIPE)
        ),
        reverse=True,
    )
    cores = [[] for _ in range(nb)]
    loads = [0] * nb
    for njp, ncp, b, T in pairs:
        cand = [c for c in range(nb) if len(cores[c]) < NSTRIPE]
        i = min(cand, key=lambda c: loads[c])
        cores[i].append((njp, ncp, b, T))
        loads[i] += njp
    for c in cores:
        c.sort(reverse=True)
    assign = [
        [(b, T, jobs[b][T]) for _, _, b, T in cores[c]] for c in range(nb)
    ]
    nj = tuple(
        max(1, max(cores[c][k][0] for c in range(nb))) for k in range(NSTRIPE)
    )
    ncap = tuple(
        max(cores[c][k][1] for c in range(nb)) for k in range(NSTRIPE)
    )
    # E-op engine split: balance V vs ACT load (costs in ns per op)
    nslot, ncaps = sum(nj), sum(ncap)
    x = int(round((800 * nslot - 400 * ncaps + 3800) / 1300.0))
    x = max(0, min(ncaps, x))
    # full-capable slots are the LAST ncap[T] of each stripe
    eact = []
    seen = 0
    for T in range(NSTRIPE):
        for j in range(nj[T]):
            if j >= nj[T] - ncap[T]:
                eact.append(seen < x)
                seen += 1
            else:
                eact.append(False)
    struct = (nj, ncap, tuple(eact))
    return struct, assign, thr


# --------------------------------------------------------------------------
# program build (per structure, cached)
# --------------------------------------------------------------------------

def _build_program(struct):
    import concourse.tile as tile
    from concourse import bacc, mybir

    dt = mybir.dt
    op = mybir.AluOpType
    af = mybir.ActivationFunctionType
    d2min_op, d2first_op, clip_op, lmin_op, lfirst_op = _get_dve_ops()
    nj, ncap, eact = struct
    nslot = sum(nj)

    nc = bacc.Bacc("TRN2", target_bir_lowering=False, debug=False)
    xt_d = nc.dram_tensor("xt", [P, W], dt.float32, kind="ExternalInput").ap()
    # per-slot scalars: [dxs, aP, dn2s, ndn2s] *nslot + [thr]
    cs_d = nc.dram_tensor("cs", [P, 4 * nslot + 1], dt.float32, kind="ExternalInput").ap()
    cdw_d = nc.dram_tensor("cdw", [P, nslot], dt.float32, kind="ExternalInput").ap()
    cbp_d = nc.dram_tensor("cbp", [P, nslot], dt.float32, kind="ExternalInput").ap()
    # one [C, 128, W] block per stripe-slot; the host reassembles into images
    out_d = nc.dram_tensor(
        "out", [NSTRIPE, C, P, W], dt.float32, kind="ExternalOutput"
    ).ap()

    with tile.TileContext(nc) as tc, ExitStack() as ctx:
        const = ctx.enter_context(tc.tile_pool(name="const", bufs=1))
        xt = const.tile_from(xt_d)
        cs = const.tile_from(cs_d)
        cbp = const.tile_from(cbp_d)
        cdw = const.tile_from(cdw_d)
        Z = const.tile([P, W], dt.float32, name="Z")
        nc.gpsimd.memset(Z[:], 0.0)

        work = ctx.enter_context(tc.tile_pool(name="work", bufs=8))
        mpool = ctx.enter_context(tc.tile_pool(name="m", bufs=16))
        opool = ctx.enter_context(tc.tile_pool(name="o", bufs=3))

        # warm the ACT function tables while const DMAs are in flight
        wu = opool.tile([P, 8], dt.float32, name="wu")
        nc.vector.memset(wu[:], 0.0)
        wu2 = opool.tile([P, 8], dt.float32, name="wu2")
        nc.scalar.activation(wu2[:], wu[:], af.Abs)
        nc.scalar.activation(wu2[:], wu[:], af.Relu)
        nc.scalar.activation(wu2[:], wu[:], af.Sqrt)

        # round-robin the stripes' jobs so all four stripes finish together
        # (8 independent min-chains keep every engine fed through the tail)
        goff = [sum(nj[:T]) for T in range(NSTRIPE)]
        chains = [[None, None] for _ in range(NSTRIPE)]

        def emit_job(T, j):
            g = goff[T] + j
            c4 = 4 * g
            Mn = mpool.tile([P, W], dt.float32, tag="M", name=f"M{g}")
            ci = j % 2
            prev = chains[T][ci]
            if j >= nj[T] - ncap[T]:
                At = work.tile([P, W], dt.float32, tag="At", name=f"At{g}")
                nc.scalar.activation(
                    At[:], xt[:], af.Abs,
                    bias=cdw[:, g : g + 1], scale=cs[:, c4 : c4 + 1],
                )
                E = work.tile([P, W], dt.float32, tag="E", name=f"E{g}")
                if eact[g]:
                    nc.scalar.activation(
                        E[:], At[:], af.Relu, bias=cs[:, c4 + 3 : c4 + 4]
                    )
                else:
                    nc.vector.tensor_scalar(
                        E[:], At[:], cs[:, c4 + 2 : c4 + 3], 0.0,
                        op0=op.subtract, op1=op.max,
                    )
                if prev is None:
                    nc.vector._custom_dve(
                        d2first_op, out=Mn[:], in0=E[:],
                        s0=cs[:, c4 + 1 : c4 + 2], s1=cbp[:, g : g + 1],
                    )
                else:
                    nc.vector._custom_dve(
                        d2min_op, out=Mn[:], in0=E[:], in1=prev[:],
                        s0=cs[:, c4 + 1 : c4 + 2], s1=cbp[:, g : g + 1],
                    )
            else:
                # line-only job: x rides Src0 (xt) — single-uOp variants
                if prev is None:
                    nc.vector._custom_dve(
                        lfirst_op, out=Mn[:], in0=xt[:],
                        s0=cs[:, c4 + 1 : c4 + 2], s1=cbp[:, g : g + 1],
                    )
                else:
                    nc.vector._custom_dve(
                        lmin_op, out=Mn[:], in0=xt[:], in1=prev[:],
                        s0=cs[:, c4 + 1 : c4 + 2], s1=cbp[:, g : g + 1],
                    )
            chains[T][ci] = Mn

        def finalize_stripe(T):
            if chains[T][1] is not None:
                M = mpool.tile([P, W], dt.float32, tag="M", name=f"Mf{T}")
                nc.vector.tensor_tensor(
                    M[:], chains[T][0][:], chains[T][1][:], op=op.min
                )
            else:
                M = chains[T][0]
            dist = opool.tile([P, W], dt.float32, tag="dist", name=f"ds{T}")
            nc.scalar.activation(dist[:], M[:], af.Sqrt)
            # cov = clip(thr - dist, 0, 1) in one fused DVE op
            cov = opool.tile([P, W], dt.float32, tag="cov", name=f"cv{T}")
            nc.vector._custom_dve(
                clip_op, out=cov[:], in0=dist[:],
                s0=cs[:, 4 * nslot : 4 * nslot + 1],
            )
            for c in range(C):
                nc.sync.dma_start(out_d[T, c, :, :], cov[:])

        for j in range(max(nj)):
            for T in range(NSTRIPE):
                if j < nj[T]:
                    emit_job(T, j)
                    if j == nj[T] - 1:
                        finalize_stripe(T)

    nc.compile()
    return nc


# --------------------------------------------------------------------------
# host coefficient tables
# --------------------------------------------------------------------------

def _prep_inputs(trajectories, struct, assign, thr):
    nj, ncap, _ = struct
    nslot = sum(nj)
    xy = np.asarray(trajectories, dtype=np.float64)[:, :, 1:3]
    nb = xy.shape[0]
    xt = np.broadcast_to(np.arange(W, dtype=np.float64), (P, W)).astype(np.float32)
    yv = np.arange(H, dtype=np.float64).reshape(NSTRIPE, P)

    geo = {}
    for b in range(nb):
        p0a, p1a, da = _segments(xy[b])
        dx, dy = da[:, 0], da[:, 1]
        dd2 = dx * dx + dy * dy
        sq = 1.0 / np.sqrt(dd2)
        dn2 = dd2 / 2.0
        c0 = dx * p0a[:, 0] + dy * p0a[:, 1]
        cP = dx * p0a[:, 1] - dy * p0a[:, 0]
        geo[b] = (dx, dy, sq, dn2, c0, cP)

    in_maps = []
    for core in range(nb):
        cs = np.zeros((P, 4 * nslot + 1))
        cdw = np.zeros((P, nslot))
        cbp = np.zeros((P, nslot))
        g = 0
        for k in range(NSTRIPE):
            b, T, myjobs = assign[core][k]
            dx, dy, sq, dn2, c0, cP = geo[b]
            cap_jobs = [s for s, cap in myjobs if cap]
            line_jobs = [s for s, cap in myjobs if not cap]
            full_start = nj[k] - ncap[k]
            slots = [None] * nj[k]
            for i, s in enumerate(cap_jobs):
                slots[full_start + i] = (s, True)
            free = list(range(full_start)) + list(
                range(full_start + len(cap_jobs), nj[k])
            )
            for s, j in zip(line_jobs, free):
                slots[j] = (s, False)
            for j in range(nj[k]):
                c4 = 4 * g
                if slots[j] is not None:
                    s, iscap = slots[j]
                    cs[:, c4 + 0] = dx[s] * sq[s]
                    cs[:, c4 + 1] = dy[s] * sq[s]
                    # E = 0 unless this is a genuine cap job (t-clamp excess
                    # provably irrelevant in this stripe otherwise)
                    if iscap:
                        cs[:, c4 + 2] = dn2[s] * sq[s]
                        cs[:, c4 + 3] = -dn2[s] * sq[s]
                    else:
                        cs[:, c4 + 2] = 1e30
                        cs[:, c4 + 3] = -1e30
                    cdw[:, g] = (dy[s] * yv[T] - (c0[s] + dn2[s])) * sq[s]
                    cbp[:, g] = (-dx[s] * yv[T] + cP[s]) * sq[s]
                else:
                    # neutral padding: d2 = 1e12, E = 0
                    cs[:, c4 + 0] = 0.0
                    cs[:, c4 + 1] = 0.0
                    cs[:, c4 + 2] = 1e30
                    cs[:, c4 + 3] = -1e30
                    cdw[:, g] = 0.0
                    cbp[:, g] = 1e6
                g += 1
        cs[:, 4 * nslot] = thr

        in_maps.append(
            {
                "xt": xt,
                "cs": cs.astype(np.float32),
                "cdw": cdw.astype(np.float32),
                "cbp": cbp.astype(np.float32),
            }
        )
    return in_maps


def kernel(**inputs):
    from concourse.bass_utils import run_bass_kernel_spmd

    images = np.asarray(inputs["images"])
    trajectories = np.asarray(inputs["trajectories"])
    line_width = inputs["line_width"]
    assert images.shape == (B, C, H, W), images.shape

    struct, assign, thr = _plan(trajectories, line_width)
    progs = _state.setdefault("progs", {})
    if struct not in progs:
        progs[struct] = _build_program(struct)
    nc = progs[struct]

    in_maps = _prep_inputs(trajectories, struct, assign, thr)
    res = run_bass_kernel_spmd(nc, in_maps, list(range(B))).results
    out = np.empty((B, C, H, W), np.float32)
    for core in range(B):
        blk = res[core]["out"]  # [NSTRIPE, C, P, W]
        for k in range(NSTRIPE):
            b, T, _ = assign[core][k]
            out[b, :, T * P : (T + 1) * P, :] = blk[k]
    return out


if __name__ == "__main__":
    rng = np.random.default_rng(0)
    ins = {
        "images": rng.standard_normal((B, C, H, W)).astype(np.float32),
        "trajectories": np.concatenate(
            [
                np.broadcast_to(np.linspace(0, 1, K, dtype=np.float32), (B, K))[..., None],
                rng.uniform(0, W - 1, (B, K, 2)).astype(np.float32),
                np.ones((B, K, 1), np.float32),
            ],
            axis=-1,
        ),
        "line_width": 3,
    }
    out = kernel(**ins)
    print(out.shape, out.dtype, out.min(), out.max())



# revision 5
# speedup vs baseline: 2.0406x; 2.0406x over previous
"""Trainium2 Bass kernel for BlittingStrokeModel (AA polyline rasterization).

Reference semantics: per batch item, 16 AA segments rasterized onto a zero
canvas via point-to-segment distance: cov = clip(lw + 0.5 - dist, 0, 1),
max over segments, broadcast to 3 channels.

Device formulation (distances scaled by 1/16 so fp16 tiles hold d^2):
    Pp  = perpendicular line distance  = (dy*x - dx*y + cP) * s / 16
    u   = along-axis coordinate        = (dx*x + dy*y - c0) * s      (px)
    E   = cap excess = relu(sigma*u - [L if sigma=+1 else 0]) / 16
    d^2 = Pp^2 + E^2 ;  M = min over segments ;  cov = clip(thr - 16*sqrt(M))

Work unit: one (image, row, column-quarter) "row-job" per segment whose
capsule (radius thr+MARG) meets that 128px quarter of that row.  All
per-segment geometry enters through PER-PARTITION scalars, so any 128
row-units pack into one [128,128] op regardless of which image/row they
come from.  Rows are sorted by job count and packed into NCOMP composite
canvases of 128 rows; program structure (ops per composite) is the
per-composite max job count — identical across cores (SPMD), with
neutral padding coefficients for idle partitions.

Per composite the min-chain runs as interleaved custom-DVE ops
(min((Idx*C0+C1)^2 + Src0^2, Src1) for cap slots with Src0 = an E tile
built by one ACT Relu; min((Src0*C0+C1)^2, Src1) with Src0 = xt for line
slots).  A balanced subset of line slots instead uses ACT Square + a
native fp16 tensor_tensor min (2x DVE packing) to equalize V and ACT
load.  Two-sided caps (both endpoint regions in range in one quarter,
rare) use an ACT Abs + tensor_scalar E build.  Finalize is batched on a
single [128, NCOMP*128] fp16 tile: merge, sqrt, clip, one DMA out.

Only a 1-channel fp16 stroke canvas leaves the device (the output is
channel-replicated and images never affect it); the host scatters rows,
casts to fp32, and broadcasts channels.
"""

import numpy as np
from contextlib import ExitStack

B, C, H, W = 8, 3, 512, 512
K = 17
NSEG = K - 1
P = 128
QW = 128          # column-quarter width
NQ = W // QW      # 4
SC = 1.0 / 16.0   # distance scale for fp16 range
MARG = 0.75
PADB = 200.0      # padding bias -> d^2 = 40000 (< fp16 max)
NCORE = 8

_state = {}


# --------------------------------------------------------------------------
# custom DVE ops
# --------------------------------------------------------------------------

def _register_dve_op(name, spec):
    import concourse.dve_ops as dve_ops
    from concourse.dve_ops import DveOp, OPS, _SUB_OPCODE_FOR_NAME, _CUSTOM_DVE_ROW_BASE
    from concourse.dve_spec import lower, _has_src1
    from concourse.dve_uop import DveOpSpec
    from concourse.dve_table_gen import dve_ver_for

    if name in _SUB_OPCODE_FOR_NAME:
        return next(o for o in OPS if o.name == name)
    row = _CUSTOM_DVE_ROW_BASE + len(OPS)
    assert row < 0x20
    _SUB_OPCODE_FOR_NAME[name] = row
    ver = dve_ver_for("TRN2")
    tmp = DveOpSpec(
        name=name, opcode=row, uops=lower(spec, ver=ver), rd1_en=_has_src1(spec)
    )
    op = DveOp(name, spec, subdim=False, uops_sha={ver: tmp.sha(ver)})
    OPS.append(op)
    dve_ops.CUSTOM_DVE_SPECS[name] = spec
    return op


def _get_dve_ops():
    if "ops" in _state:
        return _state["ops"]
    from concourse.dve_spec import Spec, Src0, Src1, C0, C1, sq, minn, maxx, Idx, Zero, One

    def _idx(in0):
        return np.arange(in0.shape[-1], dtype=np.float32)[None, :]

    d2min = _register_dve_op(
        "STROKE_D2MIN_ANT",
        Spec(
            body=minn(sq(Idx * C0 + C1) + sq(Src0), Src1),
            reference=lambda in0, in1, s0, s1, imm2: np.minimum(
                (_idx(in0) * s0 + s1) ** 2 + in0.astype(np.float32) ** 2, in1
            ).astype(np.float32),
        ),
    )
    d2first = _register_dve_op(
        "STROKE_D2_ANT",
        Spec(
            body=sq(Idx * C0 + C1) + sq(Src0),
            reference=lambda in0, in1, s0, s1, imm2: (
                (_idx(in0) * s0 + s1) ** 2 + in0.astype(np.float32) ** 2
            ).astype(np.float32),
        ),
    )
    lmin = _register_dve_op(
        "STROKE_LD2MIN_ANT",
        Spec(
            body=minn(sq(Src0 * C0 + C1), Src1),
            reference=lambda in0, in1, s0, s1, imm2: np.minimum(
                (in0.astype(np.float32) * s0 + s1) ** 2, in1
            ).astype(np.float32),
        ),
    )
    lfirst = _register_dve_op(
        "STROKE_LD2_ANT",
        Spec(
            body=sq(Src0 * C0 + C1),
            reference=lambda in0, in1, s0, s1, imm2: (
                (in0.astype(np.float32) * s0 + s1) ** 2
            ).astype(np.float32),
        ),
    )
    _state["ops"] = (d2min, d2first, lmin, lfirst)
    return _state["ops"]


# --------------------------------------------------------------------------
# host geometry
# --------------------------------------------------------------------------

def _segments(xy):
    p0, p1 = xy[:-1].copy(), xy[1:].copy()
    d = p1 - p0
    degen = (d[:, 0] ** 2 + d[:, 1] ** 2) < 1e-12
    d[degen, 0] = 1e-6
    p1 = p0 + d
    return p0, p1, d


def _row_xinterval(p0, p1, d, ys, R):
    """Per y in ys: x-interval [xlo, xhi] with dist((x,y), seg) <= R."""
    dx, dy = d
    dd = dx * dx + dy * dy
    s = 1.0 / np.sqrt(dd)
    xlo = np.full(len(ys), np.inf)
    xhi = np.full(len(ys), -np.inf)
    for px, py in (p0, p1):
        h2 = R * R - (ys - py) ** 2
        ok = h2 >= 0.0
        r = np.sqrt(np.maximum(h2, 0.0))
        xlo = np.where(ok, np.minimum(xlo, px - r), xlo)
        xhi = np.where(ok, np.maximum(xhi, px + r), xhi)
    cP = dx * p0[1] - dy * p0[0]
    if abs(dy) > 1e-12:
        for sgn in (-1.0, 1.0):
            x = (sgn * R / s + dx * ys - cP) / dy
            t = ((x - p0[0]) * dx + (ys - p0[1]) * dy) / dd
            ok = (t >= 0.0) & (t <= 1.0)
            xlo = np.where(ok, np.minimum(xlo, x), xlo)
            xhi = np.where(ok, np.maximum(xhi, x), xhi)
    return xlo, xhi


def _ray_rows_hit(pa, pb, ys, xlo, xhi, R):
    """Per y: does segment pa->pb come within R of {y} x [xlo, xhi]?"""
    d = pb - pa
    dd = float(d @ d)

    def pt_rowseg(px, py):
        cx = np.clip(px, xlo, xhi)
        return np.hypot(px - cx, py - ys)

    d1 = pt_rowseg(pa[0], pa[1])
    d2 = pt_rowseg(pb[0], pb[1])

    def pt_seg(qx, qy):
        t = np.clip(((qx - pa[0]) * d[0] + (qy - pa[1]) * d[1]) / max(dd, 1e-18), 0.0, 1.0)
        return np.hypot(pa[0] + t * d[0] - qx, pa[1] + t * d[1] - qy)

    d3 = pt_seg(xlo, ys)
    d4 = pt_seg(xhi, ys)
    best = np.minimum(np.minimum(d1, d2), np.minimum(d3, d4))
    if abs(d[1]) > 1e-15:
        t = (ys - pa[1]) / d[1]
        xc = pa[0] + t * d[0]
        cross = (t >= 0.0) & (t <= 1.0) & (xc >= xlo) & (xc <= xhi)
        best = np.where(cross, 0.0, best)
    return best <= R


def _plan(trajectories, line_width):
    thr = float(np.asarray(line_width).item()) + 0.5
    R = thr + MARG
    FAR = 1500.0
    xy_all = np.asarray(trajectories, dtype=np.float64)[:, :, 1:3]
    nb = xy_all.shape[0]

    geo = {}
    rows = {}
    ys_full = np.arange(H, dtype=np.float64)
    for b in range(nb):
        p0a, p1a, da = _segments(xy_all[b])
        gl = []
        for s in range(NSEG):
            p0, p1, d = p0a[s], p1a[s], da[s]
            dx, dy = d
            dd = dx * dx + dy * dy
            sc = 1.0 / np.sqrt(dd)
            gl.append(
                dict(
                    dx=dx, dy=dy, s=sc, L=np.sqrt(dd),
                    cP=dx * p0[1] - dy * p0[0],
                    c0u=dx * p0[0] + dy * p0[1],
                )
            )
            ylo = max(0, int(np.ceil(min(p0[1], p1[1]) - R)))
            yhi = min(H - 1, int(np.floor(max(p0[1], p1[1]) + R)))
            if ylo > yhi:
                continue
            ys = ys_full[ylo : yhi + 1]
            xlo, xhi = _row_xinterval(p0, p1, d, ys, R)
            dirv = d / max(float(np.hypot(*d)), 1e-9)
            for h in range(NQ):
                wlo, whi = h * QW - MARG, h * QW + QW - 1 + MARG
                act = (xhi >= wlo) & (xlo <= whi) & (xlo <= xhi)
                if not act.any():
                    continue
                exlo = np.maximum(xlo, wlo)
                exhi = np.minimum(xhi, whi)
                c0 = _ray_rows_hit(p0, p0 - dirv * FAR, ys, exlo, exhi, R) & act
                c1 = _ray_rows_hit(p1, p1 + dirv * FAR, ys, exlo, exhi, R) & act
                for i in np.nonzero(act)[0]:
                    y = ylo + i
                    kind = 2 if (c0[i] and c1[i]) else (-1 if c0[i] else (1 if c1[i] else 0))
                    rows.setdefault((b, y, h), []).append((s, kind))
        geo[b] = gl

    # sort rows by (njobs, ncaps, nabs) desc; blocks of 1024 -> composites
    def rkey(item):
        jl = item[1]
        return (len(jl), sum(1 for _, k in jl if k != 0), sum(1 for _, k in jl if k == 2))

    rlist = sorted(rows.items(), key=rkey, reverse=True)
    ncomp = max(1, (len(rlist) + NCORE * P - 1) // (NCORE * P))
    nj, ncap, nabs = [], [], []
    assign = [[[None] * P for _ in range(ncomp)] for _ in range(NCORE)]
    for c in range(ncomp):
        blk = rlist[c * NCORE * P : (c + 1) * NCORE * P]
        nj.append(max(rkey(it)[0] for it in blk))
        ncap.append(max(rkey(it)[1] for it in blk))
        nabs.append(max(rkey(it)[2] for it in blk))
        for i, (key, jl) in enumerate(blk):
            core, part = i % NCORE, i // NCORE
            lines = [s for s, k in jl if k == 0]
            caps1 = [(s, k) for s, k in jl if k in (-1, 1)]
            caps2 = [(s, 2) for s, k in jl if k == 2]
            assign[core][c][part] = (key[0], key[1], key[2], lines, caps1, caps2)
    struct = (tuple(nj), tuple(ncap), tuple(nabs))
    return struct, assign, thr, geo


# --------------------------------------------------------------------------
# deterministic op-mode derivation (shared by build and prep)
# --------------------------------------------------------------------------

def _derive_modes(struct):
    """Returns per-composite: number of line slots flipped to the native
    ACT-Square + fp16 TT-min path (flipped = the FIRST k line slots)."""
    nj, ncap, nabs = struct
    ncomp = len(nj)
    CUST, TTF, EACT, TSA, MERGE, TS1 = 480.0, 300.0, 450.0, 190.0, 300.0, 708.0
    BIGN = ncomp * QW
    v = sum(nj) * CUST + sum(nabs) * TSA + TS1 * (BIGN / 1280.0)
    a = sum(ncap) * EACT + 2 * (BIGN + 352) / 1.2
    merges = sum(1 for c in range(ncomp) if nj[c] >= 4)
    v += merges * MERGE
    kmax = [nj[c] - ncap[c] for c in range(ncomp)]
    k = [0] * ncomp
    flat = [c for c in range(ncomp) for _ in range(kmax[c])]
    # round-robin across composites
    order = []
    idx = [0] * ncomp
    while len(order) < len(flat):
        for c in range(ncomp):
            if idx[c] < kmax[c]:
                order.append(c)
                idx[c] += 1
    for c in order:
        if v <= a + CUST:
            break
        v += -CUST + TTF
        a += EACT
        k[c] += 1
    return k


# --------------------------------------------------------------------------
# program build (per structure, cached)
# --------------------------------------------------------------------------

def _build_program(struct):
    import concourse.tile as tile
    from concourse import bacc, mybir

    dt = mybir.dt
    op = mybir.AluOpType
    af = mybir.ActivationFunctionType
    d2min_op, d2first_op, lmin_op, lfirst_op = _get_dve_ops()
    nj, ncap, nabs = struct
    ncomp = len(nj)
    knat = _derive_modes(struct)
    G = sum(nj)
    NC = sum(ncap)
    NA = sum(nabs)
    NN = sum(knat)
    goff = [sum(nj[:c]) for c in range(ncomp)]
    coff = [sum(ncap[:c]) for c in range(ncomp)]
    aoff = [sum(nabs[:c]) for c in range(ncomp)]
    noff = [sum(knat[:c]) for c in range(ncomp)]
    BIGN = ncomp * QW

    nc = bacc.Bacc("TRN2", target_bir_lowering=False, debug=False)
    xt_d = nc.dram_tensor("xt", [P, QW], dt.float16, kind="ExternalInput").ap()
    cA_d = nc.dram_tensor("cA", [P, G], dt.float32, kind="ExternalInput").ap()
    cB_d = nc.dram_tensor("cB", [P, G], dt.float32, kind="ExternalInput").ap()
    if NC:
        rS_d = nc.dram_tensor("rS", [P, NC], dt.float32, kind="ExternalInput").ap()
        rB_d = nc.dram_tensor("rB", [P, NC], dt.float32, kind="ExternalInput").ap()
    if NA:
        hT_d = nc.dram_tensor("hT", [P, NA], dt.float32, kind="ExternalInput").ap()
    if NN:
        sA_d = nc.dram_tensor("sA", [P, NN], dt.float32, kind="ExternalInput").ap()
        sB_d = nc.dram_tensor("sB", [P, NN], dt.float32, kind="ExternalInput").ap()
    fin_d = nc.dram_tensor("fin", [P, 2], dt.float32, kind="ExternalInput").ap()
    out_d = nc.dram_tensor("out", [P, BIGN], dt.float16, kind="ExternalOutput").ap()

    with tile.TileContext(nc) as tc, ExitStack() as ctx:
        const = ctx.enter_context(tc.tile_pool(name="const", bufs=1))
        xt = const.tile_from(xt_d)
        cA = const.tile_from(cA_d)
        cB = const.tile_from(cB_d)
        rS = rB = hT = sA = sB = None
        if NC:
            rS = const.tile_from(rS_d)
            rB = const.tile_from(rB_d)
        if NA:
            hT = const.tile_from(hT_d)
        if NN:
            sA = const.tile_from(sA_d)
            sB = const.tile_from(sB_d)
        fin = const.tile_from(fin_d)

        big = ctx.enter_context(tc.tile_pool(name="big", bufs=1))
        BQ = big.tile([P, BIGN], dt.float16, name="BQ")
        BD = big.tile([P, BIGN], dt.float16, name="BD")
        BC = big.tile([P, BIGN], dt.float16, name="BC")

        epool = ctx.enter_context(tc.tile_pool(name="e", bufs=max(1, NC + NA)))
        npool = ctx.enter_context(tc.tile_pool(name="n", bufs=max(1, NN)))
        mpool = ctx.enter_context(tc.tile_pool(name="m", bufs=4 * ncomp + 4))
        wpool = ctx.enter_context(tc.tile_pool(name="w", bufs=2))

        # ACT warmup: one tiny op per function keeps table loads up front
        wu = wpool.tile([P, 8], dt.float32, name="wu")
        nc.vector.memset(wu[:], 0.0)
        wu2 = wpool.tile([P, 8], dt.float32, name="wu2")
        for fn in (af.Relu, af.Abs, af.Sqrt, af.Square):
            nc.scalar.activation(wu2[:], wu[:], fn)

        # E tiles per cap slot; native Square tiles per flipped line slot
        etiles = {c: {} for c in range(ncomp)}
        ntiles = {c: {} for c in range(ncomp)}
        for c in range(ncomp):
            for jc in range(ncap[c]):
                j = nj[c] - ncap[c] + jc
                gc = coff[c] + jc
                E = epool.tile([P, QW], dt.float16, tag="E", name=f"E{c}_{jc}")
                if jc >= ncap[c] - nabs[c]:
                    ja = aoff[c] + (jc - (ncap[c] - nabs[c]))
                    At = epool.tile([P, QW], dt.float32, tag="A32", name=f"At{c}_{jc}")
                    nc.scalar.activation(
                        At[:], xt[:], af.Abs,
                        bias=rB[:, gc : gc + 1], scale=rS[:, gc : gc + 1],
                    )
                    nc.vector.tensor_scalar(
                        E[:], At[:], hT[:, ja : ja + 1], 0.0,
                        op0=op.subtract, op1=op.max,
                    )
                else:
                    nc.scalar.activation(
                        E[:], xt[:], af.Relu,
                        bias=rB[:, gc : gc + 1], scale=rS[:, gc : gc + 1],
                    )
                etiles[c][j] = E
            for jn in range(knat[c]):
                gn = noff[c] + jn
                P2 = npool.tile([P, QW], dt.float16, tag="P2", name=f"P2_{c}_{jn}")
                nc.scalar.activation(
                    P2[:], xt[:], af.Square,
                    bias=sB[:, gn : gn + 1], scale=sA[:, gn : gn + 1],
                )
                ntiles[c][jn] = P2

        # per-composite chain schedules: custom slots then native TT folds
        # custom slots = line slots [knat[c], nj-ncap) then cap slots
        scheds = []
        for c in range(ncomp):
            cust = list(range(knat[c], nj[c] - ncap[c])) + list(range(nj[c] - ncap[c], nj[c]))
            nchain = 2 if (len(cust) + knat[c]) >= 4 else 1
            scheds.append((cust, nchain))
        chains = [[None, None] for _ in range(ncomp)]
        nsteps = max(len(s[0]) + knat[c] for c, s in enumerate(scheds))

        def emit_step(c, i):
            cust, nchain = scheds[c]
            ci = i % nchain
            prev = chains[c][ci]
            if i < len(cust):
                j = cust[i]
                g = goff[c] + j
                Mn = mpool.tile([P, QW], dt.float16, tag="M", name=f"M{c}_{i}")
                iscap = j >= nj[c] - ncap[c]
                src0 = etiles[c][j][:] if iscap else xt[:]
                o_first = d2first_op if iscap else lfirst_op
                o_min = d2min_op if iscap else lmin_op
                if prev is None:
                    nc.vector._custom_dve(
                        o_first, out=Mn[:], in0=src0,
                        s0=cA[:, g : g + 1], s1=cB[:, g : g + 1],
                    )
                else:
                    nc.vector._custom_dve(
                        o_min, out=Mn[:], in0=src0, in1=prev[:],
                        s0=cA[:, g : g + 1], s1=cB[:, g : g + 1],
                    )
                chains[c][ci] = Mn
            else:
                jn = i - len(cust)
                P2 = ntiles[c][jn]
                if prev is None:
                    chains[c][ci] = P2
                else:
                    Mn = mpool.tile([P, QW], dt.float16, tag="M", name=f"M{c}_{i}")
                    nc.vector.tensor_tensor(Mn[:], prev[:], P2[:], op=op.min)
                    chains[c][ci] = Mn

        def finalize(c):
            cust, nchain = scheds[c]
            sl = BQ[:, c * QW : (c + 1) * QW]
            if nchain == 2 and chains[c][1] is not None:
                nc.vector.tensor_tensor(sl, chains[c][0][:], chains[c][1][:], op=op.min)
            else:
                nc.vector.tensor_scalar(sl, chains[c][0][:], 1.0, 0.0, op0=mybir.AluOpType.mult, op1=op.add)

        for i in range(nsteps):
            for c in range(ncomp):
                tot = len(scheds[c][0]) + knat[c]
                if i < tot:
                    emit_step(c, i)
                    if i == tot - 1:
                        finalize(c)

        nc.scalar.activation(BD[:], BQ[:], af.Sqrt)
        # cov = min(relu(thr - 16*dist'), 1)
        nc.scalar.activation(BC[:], BD[:], af.Relu, bias=fin[:, 1:2], scale=fin[:, 0:1])
        BO = big.tile([P, BIGN], dt.float16, name="BO")
        nc.vector.tensor_scalar(BO[:], BC[:], 1.0, 0.0, op0=op.min, op1=op.add)
        nc.sync.dma_start(out_d, BO[:])

    nc.compile()
    return nc


# --------------------------------------------------------------------------
# host coefficient tables
# --------------------------------------------------------------------------

def _prep_inputs(trajectories, struct, assign, thr, geo):
    nj, ncap, nabs = struct
    ncomp = len(nj)
    knat = _derive_modes(struct)
    G = sum(nj)
    NC = sum(ncap)
    NA = sum(nabs)
    NN = sum(knat)
    goff = [sum(nj[:c]) for c in range(ncomp)]
    coff = [sum(ncap[:c]) for c in range(ncomp)]
    aoff = [sum(nabs[:c]) for c in range(ncomp)]
    noff = [sum(knat[:c]) for c in range(ncomp)]

    xt = np.broadcast_to(np.arange(QW, dtype=np.float32), (P, QW)).astype(np.float16)
    in_maps = []
    for core in range(NCORE):
        cA = np.zeros((P, G))
        cB = np.full((P, G), PADB)
        rS = np.zeros((P, max(1, NC)))
        rB = np.full((P, max(1, NC)), -1.0)
        hT = np.ones((P, max(1, NA)))
        sA = np.zeros((P, max(1, NN)))
        sB = np.full((P, max(1, NN)), PADB)
        for c in range(ncomp):
            nline_slots = nj[c] - ncap[c]
            for p in range(P):
                ent = assign[core][c][p]
                if ent is None:
                    continue
                b, y, h = ent[0], ent[1], ent[2]
                lines, caps1, caps2 = ent[3], ent[4], ent[5]
                xoff = float(h * QW)
                gl = geo[b]

                def pp_coef(s):
                    gg = gl[s]
                    a = gg["dy"] * gg["s"] * SC
                    bb = (gg["dy"] * xoff - gg["dx"] * y + gg["cP"]) * gg["s"] * SC
                    return a, bb

                # line jobs: first knat slots are native, rest custom
                for li, s in enumerate(lines):
                    a, bb = pp_coef(s)
                    if li < knat[c]:
                        sA[p, noff[c] + li] = a
                        sB[p, noff[c] + li] = bb
                    else:
                        g = goff[c] + li
                        cA[p, g] = a
                        cB[p, g] = bb
                # cap jobs fill from the END; two-sided first (into abs slots)
                for ci_, (s, kind) in enumerate(caps2 + caps1):
                    j = nj[c] - 1 - ci_
                    jc = j - nline_slots
                    g = goff[c] + j
                    gc = coff[c] + jc
                    a, bb = pp_coef(s)
                    cA[p, g] = a
                    cB[p, g] = bb
                    gg = gl[s]
                    ub = (gg["dx"] * xoff + gg["dy"] * y - gg["c0u"]) * gg["s"]
                    if jc >= ncap[c] - nabs[c]:
                        # abs flavor: At = |u - L/2| * SC ; E = max(At - h', 0)
                        rS[p, gc] = gg["dx"] * gg["s"] * SC
                        rB[p, gc] = (ub - gg["L"] / 2.0) * SC
                        hT[p, aoff[c] + (jc - (ncap[c] - nabs[c]))] = gg["L"] / 2.0 * SC
                    elif kind == 1:
                        rS[p, gc] = gg["dx"] * gg["s"] * SC
                        rB[p, gc] = (ub - gg["L"]) * SC
                    else:
                        rS[p, gc] = -gg["dx"] * gg["s"] * SC
                        rB[p, gc] = -ub * SC
        fin = np.zeros((P, 2))
        fin[:, 0] = -1.0 / SC
        fin[:, 1] = thr
        im = {
            "xt": xt,
            "cA": cA.astype(np.float32),
            "cB": cB.astype(np.float32),
            "fin": fin.astype(np.float32),
        }
        if NC:
            im["rS"] = rS.astype(np.float32)
            im["rB"] = rB.astype(np.float32)
        if NA:
            im["hT"] = hT.astype(np.float32)
        if NN:
            im["sA"] = sA.astype(np.float32)
            im["sB"] = sB.astype(np.float32)
        in_maps.append(im)
    return in_maps


def kernel(**inputs):
    from concourse.bass_utils import run_bass_kernel_spmd

    images = np.asarray(inputs["images"])
    trajectories = np.asarray(inputs["trajectories"])
    line_width = inputs["line_width"]
    assert images.shape == (B, C, H, W), images.shape

    struct, assign, thr, geo = _plan(trajectories, line_width)
    progs = _state.setdefault("progs", {})
    if struct not in progs:
        progs[struct] = _build_program(struct)
    nc = progs[struct]

    in_maps = _prep_inputs(trajectories, struct, assign, thr, geo)
    res = run_bass_kernel_spmd(nc, in_maps, list(range(NCORE))).results
    ncomp = len(struct[0])
    out = np.zeros((B, H, W), np.float32)
    for core in range(NCORE):
        blk = res[core]["out"].astype(np.float32)  # [P, ncomp*QW]
        for c in range(ncomp):
            for p in range(P):
                ent = assign[core][c][p]
                if ent is None:
                    continue
                b, y, h = ent[0], ent[1], ent[2]
                out[b, y, h * QW : (h + 1) * QW] = blk[p, c * QW : (c + 1) * QW]
    full = np.broadcast_to(out[:, None, :, :], (B, C, H, W)).copy()
    return full


if __name__ == "__main__":
    rng = np.random.default_rng(0)
    ins = {
        "images": rng.standard_normal((B, C, H, W)).astype(np.float32),
        "trajectories": np.concatenate(
            [
                np.broadcast_to(np.linspace(0, 1, K, dtype=np.float32), (B, K))[..., None],
                rng.uniform(0, W - 1, (B, K, 2)).astype(np.float32),
                np.ones((B, K, 1), np.float32),
            ],
            axis=-1,
        ),
        "line_width": 3,
    }
    out = kernel(**ins)
    print(out.shape, out.dtype, out.min(), out.max())


# revision 11
# speedup vs baseline: 2.4638x; 1.2074x over previous
"""Trainium2 Bass kernel for BlittingStrokeModel (AA polyline rasterization).

Reference semantics: per batch item, 16 AA segments rasterized onto a zero
canvas via point-to-segment distance: cov = clip(lw + 0.5 - dist, 0, 1),
max over segments, broadcast to 3 channels.

Device formulation (distances scaled by 1/16 so fp16 tiles hold d^2):
    Pp  = perpendicular line distance  = (dy*x - dx*y + cP) * s / 16
    u   = along-axis coordinate        = (dx*x + dy*y - c0) * s      (px)
    E   = cap excess = relu(sigma*u - [L if sigma=+1 else 0]) / 16
    d^2 = Pp^2 + E^2 ;  M = min over segments ;  cov = clip(thr - 16*sqrt(M))

Work unit: one (image, row, column-quarter) "row-job" per segment whose
capsule (radius thr+MARG) meets that 128px quarter of that row.  All
per-segment geometry enters through PER-PARTITION scalars, so any 128
row-units pack into one [128,128] op regardless of which image/row they
come from.  Rows are sorted by job count and packed into NCOMP composite
canvases of 128 rows; program structure (ops per composite) is the
per-composite max job count — identical across cores (SPMD), with
neutral padding coefficients for idle partitions.

Per composite the min-chain runs as interleaved custom-DVE ops
(min((Idx*C0+C1)^2 + Src0^2, Src1) for cap slots with Src0 = an E tile
built by one ACT Relu; min((Src0*C0+C1)^2, Src1) with Src0 = xt for line
slots).  A balanced subset of line slots instead uses ACT Square + a
native fp16 tensor_tensor min (2x DVE packing) to equalize V and ACT
load.  Two-sided caps (both endpoint regions in range in one quarter,
rare) use an ACT Abs + tensor_scalar E build.  Finalize is batched on a
single [128, NCOMP*128] fp16 tile: merge, sqrt, clip, one DMA out.

Only a 1-channel fp16 stroke canvas leaves the device (the output is
channel-replicated and images never affect it); the host scatters rows,
casts to fp32, and broadcasts channels.
"""

import numpy as np
from contextlib import ExitStack

B, C, H, W = 8, 3, 512, 512
K = 17
NSEG = K - 1
P = 128
QW = 128          # column-quarter width
NQ = W // QW      # 4
SC = 1.0 / 16.0   # distance scale for fp16 range
MARG = 0.75
PADB = 200.0      # padding bias -> d^2 = 40000 (< fp16 max)
NCORE = 8

_state = {}


# --------------------------------------------------------------------------
# custom DVE ops
# --------------------------------------------------------------------------

def _register_dve_op(name, spec):
    import concourse.dve_ops as dve_ops
    from concourse.dve_ops import DveOp, OPS, _SUB_OPCODE_FOR_NAME, _CUSTOM_DVE_ROW_BASE
    from concourse.dve_spec import lower, _has_src1
    from concourse.dve_uop import DveOpSpec
    from concourse.dve_table_gen import dve_ver_for

    if name in _SUB_OPCODE_FOR_NAME:
        return next(o for o in OPS if o.name == name)
    row = _CUSTOM_DVE_ROW_BASE + len(OPS)
    assert row < 0x20
    _SUB_OPCODE_FOR_NAME[name] = row
    ver = dve_ver_for("TRN2")
    tmp = DveOpSpec(
        name=name, opcode=row, uops=lower(spec, ver=ver), rd1_en=_has_src1(spec)
    )
    op = DveOp(name, spec, subdim=False, uops_sha={ver: tmp.sha(ver)})
    OPS.append(op)
    dve_ops.CUSTOM_DVE_SPECS[name] = spec
    return op


def _get_dve_ops():
    if "ops" in _state:
        return _state["ops"]
    from concourse.dve_spec import Spec, Src0, Src1, C0, C1, sq, minn, maxx, Idx, Zero, One

    def _idx(in0):
        return np.arange(in0.shape[-1], dtype=np.float32)[None, :]

    d2min = _register_dve_op(
        "STROKE_D2MIN_ANT",
        Spec(
            body=minn(sq(Idx * C0 + C1) + sq(Src0), Src1),
            reference=lambda in0, in1, s0, s1, imm2: np.minimum(
                (_idx(in0) * s0 + s1) ** 2 + in0.astype(np.float32) ** 2, in1
            ).astype(np.float32),
        ),
    )
    d2first = _register_dve_op(
        "STROKE_D2_ANT",
        Spec(
            body=sq(Idx * C0 + C1) + sq(Src0),
            reference=lambda in0, in1, s0, s1, imm2: (
                (_idx(in0) * s0 + s1) ** 2 + in0.astype(np.float32) ** 2
            ).astype(np.float32),
        ),
    )
    lmin = _register_dve_op(
        "STROKE_LD2MIN_ANT",
        Spec(
            body=minn(sq(Src0 * C0 + C1), Src1),
            reference=lambda in0, in1, s0, s1, imm2: np.minimum(
                (in0.astype(np.float32) * s0 + s1) ** 2, in1
            ).astype(np.float32),
        ),
    )
    lfirst = _register_dve_op(
        "STROKE_LD2_ANT",
        Spec(
            body=sq(Src0 * C0 + C1),
            reference=lambda in0, in1, s0, s1, imm2: (
                (in0.astype(np.float32) * s0 + s1) ** 2
            ).astype(np.float32),
        ),
    )
    clips = _register_dve_op(
        "STROKE_CLIPS_ANT",
        Spec(
            body=minn(maxx(Src0 * C0 + C1, Zero), One),
            reference=lambda in0, in1, s0, s1, imm2: np.minimum(
                np.maximum(in0.astype(np.float32) * s0 + s1, 0.0), 1.0
            ).astype(np.float32),
        ),
    )
    _state["ops"] = (d2min, d2first, lmin, lfirst, clips)
    return _state["ops"]


# --------------------------------------------------------------------------
# host geometry
# --------------------------------------------------------------------------

def _segments(xy):
    p0, p1 = xy[:-1].copy(), xy[1:].copy()
    d = p1 - p0
    degen = (d[:, 0] ** 2 + d[:, 1] ** 2) < 1e-12
    d[degen, 0] = 1e-6
    p1 = p0 + d
    return p0, p1, d


def _row_xinterval(p0, p1, d, ys, R):
    """Per y in ys: x-interval [xlo, xhi] with dist((x,y), seg) <= R."""
    dx, dy = d
    dd = dx * dx + dy * dy
    s = 1.0 / np.sqrt(dd)
    xlo = np.full(len(ys), np.inf)
    xhi = np.full(len(ys), -np.inf)
    for px, py in (p0, p1):
        h2 = R * R - (ys - py) ** 2
        ok = h2 >= 0.0
        r = np.sqrt(np.maximum(h2, 0.0))
        xlo = np.where(ok, np.minimum(xlo, px - r), xlo)
        xhi = np.where(ok, np.maximum(xhi, px + r), xhi)
    cP = dx * p0[1] - dy * p0[0]
    if abs(dy) > 1e-12:
        for sgn in (-1.0, 1.0):
            x = (sgn * R / s + dx * ys - cP) / dy
            t = ((x - p0[0]) * dx + (ys - p0[1]) * dy) / dd
            ok = (t >= 0.0) & (t <= 1.0)
            xlo = np.where(ok, np.minimum(xlo, x), xlo)
            xhi = np.where(ok, np.maximum(xhi, x), xhi)
    return xlo, xhi


def _ray_rows_hit(pa, pb, ys, xlo, xhi, R):
    """Per y: does segment pa->pb come within R of {y} x [xlo, xhi]?"""
    d = pb - pa
    dd = float(d @ d)

    def pt_rowseg(px, py):
        cx = np.clip(px, xlo, xhi)
        return np.hypot(px - cx, py - ys)

    d1 = pt_rowseg(pa[0], pa[1])
    d2 = pt_rowseg(pb[0], pb[1])

    def pt_seg(qx, qy):
        t = np.clip(((qx - pa[0]) * d[0] + (qy - pa[1]) * d[1]) / max(dd, 1e-18), 0.0, 1.0)
        return np.hypot(pa[0] + t * d[0] - qx, pa[1] + t * d[1] - qy)

    d3 = pt_seg(xlo, ys)
    d4 = pt_seg(xhi, ys)
    best = np.minimum(np.minimum(d1, d2), np.minimum(d3, d4))
    if abs(d[1]) > 1e-15:
        t = (ys - pa[1]) / d[1]
        xc = pa[0] + t * d[0]
        cross = (t >= 0.0) & (t <= 1.0) & (xc >= xlo) & (xc <= xhi)
        best = np.where(cross, 0.0, best)
    return best <= R


def _plan(trajectories, line_width):
    thr = float(np.asarray(line_width).item()) + 0.5
    R = thr + MARG
    FAR = 1500.0
    xy_all = np.asarray(trajectories, dtype=np.float64)[:, :, 1:3]
    nb = xy_all.shape[0]

    geo = {}
    rows = {}
    ys_full = np.arange(H, dtype=np.float64)
    for b in range(nb):
        p0a, p1a, da = _segments(xy_all[b])
        gl = []
        for s in range(NSEG):
            p0, p1, d = p0a[s], p1a[s], da[s]
            dx, dy = d
            dd = dx * dx + dy * dy
            sc = 1.0 / np.sqrt(dd)
            gl.append(
                dict(
                    dx=dx, dy=dy, s=sc, L=np.sqrt(dd),
                    cP=dx * p0[1] - dy * p0[0],
                    c0u=dx * p0[0] + dy * p0[1],
                )
            )
            ylo = max(0, int(np.ceil(min(p0[1], p1[1]) - R)))
            yhi = min(H - 1, int(np.floor(max(p0[1], p1[1]) + R)))
            if ylo > yhi:
                continue
            ys = ys_full[ylo : yhi + 1]
            xlo, xhi = _row_xinterval(p0, p1, d, ys, R)
            dirv = d / max(float(np.hypot(*d)), 1e-9)
            for h in range(NQ):
                wlo, whi = h * QW - MARG, h * QW + QW - 1 + MARG
                act = (xhi >= wlo) & (xlo <= whi) & (xlo <= xhi)
                if not act.any():
                    continue
                exlo = np.maximum(xlo, wlo)
                exhi = np.minimum(xhi, whi)
                c0 = _ray_rows_hit(p0, p0 - dirv * FAR, ys, exlo, exhi, R) & act
                c1 = _ray_rows_hit(p1, p1 + dirv * FAR, ys, exlo, exhi, R) & act
                for i in np.nonzero(act)[0]:
                    y = ylo + i
                    kind = 2 if (c0[i] and c1[i]) else (-1 if c0[i] else (1 if c1[i] else 0))
                    rows.setdefault((b, y, h), []).append((s, kind))
        geo[b] = gl

    # sort rows by (njobs, ncaps, nabs) desc; blocks of 1024 -> composites
    def rkey(item):
        jl = item[1]
        return (len(jl), sum(1 for _, k in jl if k != 0), sum(1 for _, k in jl if k == 2))

    rlist = sorted(rows.items(), key=rkey, reverse=True)
    ncomp = max(1, (len(rlist) + NCORE * P - 1) // (NCORE * P))
    nj, ncap, nabs = [], [], []
    assign = [[[None] * P for _ in range(ncomp)] for _ in range(NCORE)]
    for c in range(ncomp):
        blk = rlist[c * NCORE * P : (c + 1) * NCORE * P]
        nj.append(max(rkey(it)[0] for it in blk))
        ncap.append(max(rkey(it)[1] for it in blk))
        nabs.append(max(rkey(it)[2] for it in blk))
        for i, (key, jl) in enumerate(blk):
            core, part = i % NCORE, i // NCORE
            lines = [s for s, k in jl if k == 0]
            caps1 = [(s, k) for s, k in jl if k in (-1, 1)]
            caps2 = [(s, 2) for s, k in jl if k == 2]
            assign[core][c][part] = (key[0], key[1], key[2], lines, caps1, caps2)
    struct = (tuple(nj), tuple(ncap), tuple(nabs))
    return struct, assign, thr, geo


# --------------------------------------------------------------------------
# deterministic op-mode derivation (shared by build and prep)
# --------------------------------------------------------------------------

def _derive_modes(struct):
    """Returns per-composite: number of line slots flipped to the native
    ACT-Square + fp16 TT-min path (flipped = the FIRST k line slots).
    Cost constants are measured on HW (incl. per-op semaphore overhead)."""
    nj, ncap, nabs = struct
    ncomp = len(nj)
    CUST, TTF, EACT, TSA, MERGE = 535.0, 346.0, 600.0, 250.0, 346.0
    BIGN = ncomp * QW
    # V: customs + abs-TS + merges + 2 batched clips; ACT: E relus + 2 sqrts
    v = sum(nj) * CUST + sum(nabs) * TSA + 2 * ((BIGN / 2 + 256) / 0.96 + 130)
    a = sum(ncap) * EACT + 2 * ((BIGN / 2 + 352) / 1.2 + 123) + 2566
    merges = sum(1 for c in range(ncomp) if nj[c] >= 4)
    v += merges * MERGE
    kmax = [nj[c] - ncap[c] for c in range(ncomp)]
    k = [0] * ncomp
    order = []
    idx = [0] * ncomp
    while len(order) < sum(kmax):
        for c in range(ncomp):
            if idx[c] < kmax[c]:
                order.append(c)
                idx[c] += 1
    for c in order:
        if v <= a + CUST:
            break
        v += -CUST + TTF
        a += EACT
        k[c] += 1
    return k


# --------------------------------------------------------------------------
# program build (per structure, cached)
# --------------------------------------------------------------------------

def _tab_layout(struct, knat):
    nj, ncap, nabs = struct
    G, NC, NA, NN = sum(nj), sum(ncap), sum(nabs), sum(knat)
    o = {}
    o["cA"] = 0
    o["cB"] = G
    o["rS"] = 2 * G
    o["rB"] = 2 * G + NC
    o["hT"] = 2 * G + 2 * NC
    o["sA"] = 2 * G + 2 * NC + NA
    o["sB"] = o["sA"] + NN
    o["fin"] = o["sB"] + NN
    o["TB"] = o["fin"] + 2
    return o


def _build_program(struct):
    import concourse.tile as tile
    from concourse import bacc, mybir

    dt = mybir.dt
    op = mybir.AluOpType
    af = mybir.ActivationFunctionType
    d2min_op, d2first_op, lmin_op, lfirst_op, clips_op = _get_dve_ops()
    nj, ncap, nabs = struct
    ncomp = len(nj)
    knat = _derive_modes(struct)
    goff = [sum(nj[:c]) for c in range(ncomp)]
    coff = [sum(ncap[:c]) for c in range(ncomp)]
    aoff = [sum(nabs[:c]) for c in range(ncomp)]
    noff = [sum(knat[:c]) for c in range(ncomp)]
    NC, NA, NN = sum(ncap), sum(nabs), sum(knat)
    BIGN = ncomp * QW
    L = _tab_layout(struct, knat)

    nc = bacc.Bacc("TRN2", target_bir_lowering=False, debug=False)
    xt_d = nc.dram_tensor("xt", [P, QW], dt.float16, kind="ExternalInput").ap()
    tb_d = nc.dram_tensor("tb", [P, L["TB"]], dt.float32, kind="ExternalInput").ap()
    out_d = nc.dram_tensor("out", [P, BIGN], dt.float16, kind="ExternalOutput").ap()

    with tile.TileContext(nc) as tc, ExitStack() as ctx:
        const = ctx.enter_context(tc.tile_pool(name="const", bufs=1))
        xt = const.tile_from(xt_d)
        tb = const.tile_from(tb_d)

        def T(key, i):
            offi = L[key] + i
            return tb[:, offi : offi + 1]

        big = ctx.enter_context(tc.tile_pool(name="big", bufs=1))
        BQ = big.tile([P, BIGN], dt.float16, name="BQ")
        BD = big.tile([P, BIGN], dt.float16, name="BD")
        BO = big.tile([P, BIGN], dt.float16, name="BO")

        epool = ctx.enter_context(tc.tile_pool(name="e", bufs=max(1, NC + NA)))
        npool = ctx.enter_context(tc.tile_pool(name="n", bufs=max(1, NN)))
        mpool = ctx.enter_context(tc.tile_pool(name="m", bufs=4 * ncomp + 4))
        wpool = ctx.enter_context(tc.tile_pool(name="w", bufs=2))

        # ACT warmup: one tiny op per function keeps table loads up front
        wu = wpool.tile([P, 8], dt.float32, name="wu")
        nc.vector.memset(wu[:], 0.0)
        wu2 = wpool.tile([P, 8], dt.float32, name="wu2")
        for fn in (af.Relu, af.Abs, af.Sqrt, af.Square):
            nc.scalar.activation(wu2[:], wu[:], fn)

        # chain schedules: line customs, then native folds, then caps
        scheds, nchains = [], []
        for c in range(ncomp):
            steps = (
                [("L", j) for j in range(knat[c], nj[c] - ncap[c])]
                + [("F", jn) for jn in range(knat[c])]
                + [("C", j) for j in range(nj[c] - ncap[c], nj[c])]
            )
            scheds.append(steps)
            nchains.append(2 if nj[c] >= 4 else 1)

        # ACT producers: Squares first (folds consume before caps), then Es
        ntiles = {c: {} for c in range(ncomp)}
        etiles = {c: {} for c in range(ncomp)}
        for c in range(ncomp):
            for jn in range(knat[c]):
                gn = noff[c] + jn
                # a single-step composite writes its Square straight to BQ
                if len(scheds[c]) == 1 and nchains[c] == 1 and scheds[c][0][0] == "F":
                    ntiles[c][jn] = None
                    nc.scalar.activation(
                        BQ[:, c * QW : (c + 1) * QW], xt[:], af.Square,
                        bias=T("sB", gn), scale=T("sA", gn),
                    )
                    continue
                P2 = npool.tile([P, QW], dt.float16, tag="P2", name=f"P2_{c}_{jn}")
                nc.scalar.activation(
                    P2[:], xt[:], af.Square, bias=T("sB", gn), scale=T("sA", gn)
                )
                ntiles[c][jn] = P2
        for c in range(ncomp):
            for jc in range(ncap[c]):
                j = nj[c] - ncap[c] + jc
                gc = coff[c] + jc
                E = epool.tile([P, QW], dt.float16, tag="E", name=f"E{c}_{jc}")
                if jc >= ncap[c] - nabs[c]:
                    ja = aoff[c] + (jc - (ncap[c] - nabs[c]))
                    At = epool.tile([P, QW], dt.float32, tag="A32", name=f"At{c}_{jc}")
                    nc.scalar.activation(
                        At[:], xt[:], af.Abs, bias=T("rB", gc), scale=T("rS", gc)
                    )
                    nc.vector.tensor_scalar(
                        E[:], At[:], T("hT", ja), 0.0, op0=op.subtract, op1=op.max
                    )
                else:
                    nc.scalar.activation(
                        E[:], xt[:], af.Relu, bias=T("rB", gc), scale=T("rS", gc)
                    )
                etiles[c][j] = E

        chains = [[None, None] for _ in range(ncomp)]
        emitted = [0] * ncomp

        def emit_step(c, i):
            kind, j = scheds[c][i]
            ci = i % nchains[c]
            prev = chains[c][ci]  # AP or None
            last = i == len(scheds[c]) - 1 and nchains[c] == 1
            sl = BQ[:, c * QW : (c + 1) * QW]
            if kind in ("L", "C"):
                g = goff[c] + j
                out = sl if last else mpool.tile([P, QW], dt.float16, tag="M", name=f"M{c}_{i}")[:]
                iscap = kind == "C"
                src0 = etiles[c][j][:] if iscap else xt[:]
                if prev is None:
                    nc.vector._custom_dve(
                        d2first_op if iscap else lfirst_op,
                        out=out, in0=src0, s0=T("cA", g), s1=T("cB", g),
                    )
                else:
                    nc.vector._custom_dve(
                        d2min_op if iscap else lmin_op,
                        out=out, in0=src0, in1=prev,
                        s0=T("cA", g), s1=T("cB", g),
                    )
                chains[c][ci] = out
            else:
                P2 = ntiles[c][j]
                if P2 is None:  # already written straight to BQ
                    chains[c][ci] = sl
                    return
                if prev is None:
                    chains[c][ci] = P2[:]
                else:
                    out = sl if last else mpool.tile([P, QW], dt.float16, tag="M", name=f"M{c}_{i}")[:]
                    nc.vector.tensor_tensor(out, prev, P2[:], op=op.min)
                    chains[c][ci] = out

        # V emission: all line-customs, then folds, then caps (round-robin)
        for phase in ("L", "F", "C"):
            progressed = True
            while progressed:
                progressed = False
                for c in range(ncomp):
                    i = emitted[c]
                    if i < len(scheds[c]) and scheds[c][i][0] == phase:
                        emit_step(c, i)
                        emitted[c] += 1
                        progressed = True

        # merges + split finalize: light half (high c) first
        halves = [list(range(ncomp // 2, ncomp)), list(range(ncomp // 2))]
        for hi, comps in enumerate(halves):
            for c in comps:
                if nchains[c] == 2 and chains[c][1] is not None:
                    nc.vector.tensor_tensor(
                        BQ[:, c * QW : (c + 1) * QW], chains[c][0], chains[c][1], op=op.min
                    )
            lo = min(comps) * QW
            hhi = (max(comps) + 1) * QW
            nc.scalar.activation(BD[:, lo:hhi], BQ[:, lo:hhi], af.Sqrt)
            nc.vector._custom_dve(
                clips_op, out=BO[:, lo:hhi], in0=BD[:, lo:hhi],
                s0=T("fin", 0), s1=T("fin", 1),
            )
            nc.sync.dma_start(out_d[:, lo:hhi], BO[:, lo:hhi])

    nc.compile()
    return nc


# --------------------------------------------------------------------------
# host coefficient tables
# --------------------------------------------------------------------------

def _prep_inputs(trajectories, struct, assign, thr, geo):
    nj, ncap, nabs = struct
    ncomp = len(nj)
    knat = _derive_modes(struct)
    G = sum(nj)
    NC = sum(ncap)
    NA = sum(nabs)
    NN = sum(knat)
    goff = [sum(nj[:c]) for c in range(ncomp)]
    coff = [sum(ncap[:c]) for c in range(ncomp)]
    aoff = [sum(nabs[:c]) for c in range(ncomp)]
    noff = [sum(knat[:c]) for c in range(ncomp)]
    L = _tab_layout(struct, knat)

    xt = np.broadcast_to(np.arange(QW, dtype=np.float32), (P, QW)).astype(np.float16)
    in_maps = []
    for core in range(NCORE):
        cA = np.zeros((P, G))
        cB = np.full((P, G), PADB)
        rS = np.zeros((P, max(1, NC)))
        rB = np.full((P, max(1, NC)), -1.0)
        hT = np.ones((P, max(1, NA)))
        sA = np.zeros((P, max(1, NN)))
        sB = np.full((P, max(1, NN)), PADB)
        for c in range(ncomp):
            nline_slots = nj[c] - ncap[c]
            for p in range(P):
                ent = assign[core][c][p]
                if ent is None:
                    continue
                b, y, h = ent[0], ent[1], ent[2]
                lines, caps1, caps2 = ent[3], ent[4], ent[5]
                xoff = float(h * QW)
                gl = geo[b]

                def pp_coef(s):
                    gg = gl[s]
                    a = gg["dy"] * gg["s"] * SC
                    bb = (gg["dy"] * xoff - gg["dx"] * y + gg["cP"]) * gg["s"] * SC
                    return a, bb

                # line jobs: first knat slots are native, rest custom
                for li, s in enumerate(lines):
                    a, bb = pp_coef(s)
                    if li < knat[c]:
                        sA[p, noff[c] + li] = a
                        sB[p, noff[c] + li] = bb
                    else:
                        g = goff[c] + li
                        cA[p, g] = a
                        cB[p, g] = bb
                # cap jobs fill from the END; two-sided first (into abs slots)
                for ci_, (s, kind) in enumerate(caps2 + caps1):
                    j = nj[c] - 1 - ci_
                    jc = j - nline_slots
                    g = goff[c] + j
                    gc = coff[c] + jc
                    a, bb = pp_coef(s)
                    cA[p, g] = a
                    cB[p, g] = bb
                    gg = gl[s]
                    ub = (gg["dx"] * xoff + gg["dy"] * y - gg["c0u"]) * gg["s"]
                    if jc >= ncap[c] - nabs[c]:
                        # abs flavor: At = |u - L/2| * SC ; E = max(At - h', 0)
                        rS[p, gc] = gg["dx"] * gg["s"] * SC
                        rB[p, gc] = (ub - gg["L"] / 2.0) * SC
                        hT[p, aoff[c] + (jc - (ncap[c] - nabs[c]))] = gg["L"] / 2.0 * SC
                    elif kind == 1:
                        rS[p, gc] = gg["dx"] * gg["s"] * SC
                        rB[p, gc] = (ub - gg["L"]) * SC
                    else:
                        rS[p, gc] = -gg["dx"] * gg["s"] * SC
                        rB[p, gc] = -ub * SC
        tb = np.zeros((P, L["TB"]))
        tb[:, L["cA"] : L["cA"] + G] = cA
        tb[:, L["cB"] : L["cB"] + G] = cB
        if NC:
            tb[:, L["rS"] : L["rS"] + NC] = rS
            tb[:, L["rB"] : L["rB"] + NC] = rB
        if NA:
            tb[:, L["hT"] : L["hT"] + NA] = hT
        if NN:
            tb[:, L["sA"] : L["sA"] + NN] = sA
            tb[:, L["sB"] : L["sB"] + NN] = sB
        tb[:, L["fin"]] = -1.0 / SC
        tb[:, L["fin"] + 1] = thr
        in_maps.append({"xt": xt, "tb": tb.astype(np.float32)})
    return in_maps


def kernel(**inputs):
    from concourse.bass_utils import run_bass_kernel_spmd

    images = np.asarray(inputs["images"])
    trajectories = np.asarray(inputs["trajectories"])
    line_width = inputs["line_width"]
    assert images.shape == (B, C, H, W), images.shape

    struct, assign, thr, geo = _plan(trajectories, line_width)
    progs = _state.setdefault("progs", {})
    if struct not in progs:
        progs[struct] = _build_program(struct)
    nc = progs[struct]

    in_maps = _prep_inputs(trajectories, struct, assign, thr, geo)
    res = run_bass_kernel_spmd(nc, in_maps, list(range(NCORE))).results
    ncomp = len(struct[0])
    out = np.zeros((B, H, W), np.float32)
    for core in range(NCORE):
        blk = res[core]["out"].astype(np.float32)  # [P, ncomp*QW]
        for c in range(ncomp):
            for p in range(P):
                ent = assign[core][c][p]
                if ent is None:
                    continue
                b, y, h = ent[0], ent[1], ent[2]
                out[b, y, h * QW : (h + 1) * QW] = blk[p, c * QW : (c + 1) * QW]
    full = np.broadcast_to(out[:, None, :, :], (B, C, H, W)).copy()
    return full


if __name__ == "__main__":
    rng = np.random.default_rng(0)
    ins = {
        "images": rng.standard_normal((B, C, H, W)).astype(np.float32),
        "trajectories": np.concatenate(
            [
                np.broadcast_to(np.linspace(0, 1, K, dtype=np.float32), (B, K))[..., None],
                rng.uniform(0, W - 1, (B, K, 2)).astype(np.float32),
                np.ones((B, K, 1), np.float32),
            ],
            axis=-1,
        ),
        "line_width": 3,
    }
    out = kernel(**ins)
    print(out.shape, out.dtype, out.min(), out.max())
